# revision 22
# baseline (speedup 1.0000x reference)
"""Trainium2 Bass kernel for the LELoss problem (raw Bass, 8-core SPMD).

loss = mean_b ||x_b - dec_b||^2
     + 1.1 * mean_b ||enc_b - (lat @ rsrA.T)_b||^2
     + 0.1 * mean((rsrA.T @ rsrA - I)^2)

(The knn/cdist/topk in the original module is dead code - its result is never
used - so the returned loss reduces to the three terms above.)

Per-core algebra (batch shard of R=1024 rows):
  sum||enc - lat@A.T||^2 = sum(enc^2) - 2*sum(M .* A) + sum(L .* G0)
      with M = enc.T @ lat [E,I], L = lat.T @ lat [I,I], G0 = A.T @ A [I,I]
  sum((G0 - I)^2) = sum(G0^2) - 2*sum(A^2) + I_dim
All partial sums land in columns of a [128,16] SBUF accumulator S which is
DMA'd out per core; the host collapses partitions/cores and applies weights.

DMA strategy (v7): ONE HWDGE ring (SP) carries the whole stream - the 16
SDMA engines hit the same ~427GB/s from one ring as from two, completion
sems fire in strict FIFO right behind the data, and the ACT engine is freed
for compute. Only [128, C] tiles (other partition counts break the
16-engine descriptor spray). Pair-adjacent order x_t, dec_t: three
[128,2048] row-pair-packed tiles (8KB descriptors, which also evens out the
~12% slower SDMA engine 15), the enc/lat pack right after the first pair
(so the PE matmuls and reductions run mid-stream), then a [128,1024] pair
and two [128,512] column-half pairs so the post-stream tail is only ~1.1us
(subtract 320/192-split between DVE and ACT). A dummy 1-element activation
pre-triggers the 1.3us ACT table load, and the out DMA's completion is left
to the NEFF teardown (~7us sem-reset postamble) instead of a semaphore wait.
"""

import contextlib

import numpy as np

try:
    import concourse.bass as bass
except ImportError:  # pragma: no cover - grading env fallback
    import sys

    sys.path.insert(0, "/opt/trn_rl_repo")
    import concourse.bass as bass

from concourse import mybir
from concourse.bass_utils import run_bass_kernel_spmd

N_CORES = 8
B, D, E, I = 8192, 1024, 128, 20
R = B // N_CORES  # rows per core = 1024
P = 128  # SBUF partitions
S_COLS = 16
F32 = mybir.dt.float32

RT = R // P  # enc pack row groups = 8
ENC_W = RT * E  # 1024 cols of packed enc
LAT_W = RT * I  # 160 cols of packed lat
PACK_W = ENC_W + LAT_W + I  # 1204

# stream tiles: (row0, nrows, col0, ncols). 256-row tiles land as
# [128, 2048] row-pair-packed views with 8KB descriptors.
TILES = [
    (0, 256, 0, D),      # [128,2048]
    (256, 256, 0, D),    # [128,2048]
    (512, 256, 0, D),    # [128,2048]
    (768, 128, 0, D),    # [128,1024]
    (896, 128, 0, 512),  # [128,512]
    (896, 128, 512, 512),  # [128,512]
]
NT = len(TILES)
LAST = NT - 1
SQ_TOTAL = NT + 2  # per-tile squares (NT-1 + tail-B) + enc^2 + rsrA^2

# column split of the final [128,512] tile: ACT squares CB cols,
# DVE STT-squares the remaining CA cols (DVE also does both subtracts)
CB = 320
CA = 512 - CB  # 192

WAIT_OUT = False  # wait for the out DMA's completion semaphore before ending

TRACE = False
LAST_RESULT = None

_NC = None


def _build_nc():
    nc = bass.Bass()
    x = nc.dram_tensor("x", [R, D], F32, kind="ExternalInput")
    dec = nc.dram_tensor("dec", [R, D], F32, kind="ExternalInput")
    pack = nc.dram_tensor("pack", [P, PACK_W], F32, kind="ExternalInput")
    out = nc.dram_tensor("out", [P, S_COLS], F32, kind="ExternalOutput")

    Square = mybir.ActivationFunctionType.Square
    mult = mybir.AluOpType.mult
    bypass = mybir.AluOpType.bypass

    def tile_src(dram, t):
        r0, rn, c0, cn = TILES[t]
        ap = dram[r0 : r0 + rn, c0 : c0 + cn]
        if rn == 2 * P:
            ap = ap.rearrange("(p two) d -> p (two d)", two=2)
        return ap

    def sbuf_cols(t):
        r0, rn, c0, cn = TILES[t]
        return cn * (rn // P)

    ctx = contextlib.ExitStack()
    with ctx:
        xb = [
            ctx.enter_context(nc.sbuf_tensor(f"xb{t}", [P, sbuf_cols(t)], F32))
            for t in range(NT)
        ]
        db = [
            ctx.enter_context(nc.sbuf_tensor(f"db{t}", [P, sbuf_cols(t)], F32))
            for t in range(NT)
        ]
        small_sb = ctx.enter_context(nc.sbuf_tensor([P, PACK_W], F32))
        S = ctx.enter_context(nc.sbuf_tensor([P, S_COLS], F32))
        G_sb = ctx.enter_context(nc.sbuf_tensor([I, I], F32))
        scr_m = ctx.enter_context(nc.sbuf_tensor([E, I], F32))
        scr_i = ctx.enter_context(nc.sbuf_tensor([I, I], F32))
        scr_a = ctx.enter_context(nc.sbuf_tensor([E, I], F32))
        scr_e = ctx.enter_context(nc.sbuf_tensor([P, ENC_W], F32))
        dummy = ctx.enter_context(nc.sbuf_tensor([P, 2], F32))

        psum_M = ctx.enter_context(nc.psum_tensor([E, I], F32))
        psum_L = ctx.enter_context(nc.psum_tensor([I, I], F32))
        psum_G = ctx.enter_context(nc.psum_tensor([I, I], F32))

        # pair sems: tile t complete when s_x[t] >= 32 (16 from x, 16 from dec)
        s_x = [ctx.enter_context(nc.semaphore(f"s_x{t}")) for t in range(NT)]
        s_packE = ctx.enter_context(nc.semaphore("s_packE"))
        s_init = ctx.enter_context(nc.semaphore("s_init"))
        s_sub = ctx.enter_context(nc.semaphore("s_sub"))
        s_sq = ctx.enter_context(nc.semaphore("s_sq"))
        s_pe = ctx.enter_context(nc.semaphore("s_pe"))
        s_vfin = ctx.enter_context(nc.semaphore("s_vfin"))
        s_out = ctx.enter_context(nc.semaphore("s_out"))

        block = ctx.enter_context(nc.Block())

        def enc_t(t):
            return small_sb[:, t * E : (t + 1) * E]

        def lat_t(t):
            return small_sb[:, ENC_W + t * I : ENC_W + (t + 1) * I]

        rsra_sb = small_sb[:, ENC_W + LAT_W : PACK_W]

        @block.sync
        def _(sync):
            def pair(t):
                sync.dma_start(out=xb[t][:, :], in_=tile_src(x, t)).then_inc(
                    s_x[t], 16
                )
                sync.dma_start(out=db[t][:, :], in_=tile_src(dec, t)).then_inc(
                    s_x[t], 16
                )

            pair(0)
            sync.dma_start(out=small_sb[:, :], in_=pack[:, :]).then_inc(s_packE, 16)
            for t in range(1, NT):
                pair(t)


        @block.scalar
        def _(scalar):
            # pre-trigger the ACT function-table load while the stream ramps
            nc.scalar.activation(out=dummy[:, 0:1], in_=dummy[:, 1:2], func=Square)
            # squares of the streamed differences (tiles 0..NT-2)
            scalar.wait_ge(s_init, 1)
            for t in range(NT - 1):
                scalar.wait_ge(s_sub, t + 1)
                nc.scalar.activation(
                    out=db[t][:, :], in_=xb[t][:, :], func=Square,
                    accum_out=S[:, t : t + 1],
                ).then_inc(s_sq, 1)
                if t == 1:
                    scalar.wait_ge(s_packE, 16)
                    nc.scalar.activation(
                        out=scr_e[:, :], in_=small_sb[:, 0:ENC_W], func=Square,
                        accum_out=S[:, 7:8],
                    ).then_inc(s_sq, 1)
                    nc.scalar.activation(
                        out=scr_a[:, :], in_=rsra_sb, func=Square,
                        accum_out=S[:E, 8:9],
                    ).then_inc(s_sq, 1)
            # tail: square the CB-column part as soon as the DVE subtracted it
            scalar.wait_ge(s_sub, NT)
            nc.scalar.activation(
                out=db[LAST][:, 0:CB], in_=xb[LAST][:, 0:CB], func=Square,
                accum_out=S[:, 5:6],
            ).then_inc(s_sq, 1)
            # ship the accumulator: all ACT-written S columns are ordered
            # before this on this engine; DVE's columns via s_vfin
            scalar.wait_ge(s_vfin, 2)
            scalar.dma_start(out=out[:, :], in_=S[:, :]).then_inc(s_out, 16)
            if WAIT_OUT:
                scalar.wait_ge(s_out, 16)

        @block.vector
        def _(vector):
            nc.vector.memset(S[:, :], 0.0).then_inc(s_init, 1)
            # the big stream: d = x - dec, in place
            for t in range(NT - 1):
                vector.wait_ge(s_x[t], 32)
                nc.vector.tensor_sub(xb[t][:, :], xb[t][:, :], db[t][:, :]).then_inc(
                    s_sub, 1
                )
                if t == 1:
                    # tiny fused reductions over the PCA/proj matmul results,
                    # mid-stream in a gap between subtracts
                    vector.wait_ge(s_pe, 1)
                    nc.vector.tensor_copy(G_sb[:, :], psum_G[:, :])
                    nc.vector.scalar_tensor_tensor(
                        out=scr_m[:, :], in0=psum_M[:, :], scalar=1.0, in1=rsra_sb,
                        op0=bypass, op1=mult, accum_out=S[:E, 9:10],
                    )
                    nc.vector.scalar_tensor_tensor(
                        out=scr_i[:, :], in0=psum_L[:, :], scalar=1.0, in1=G_sb[:, :],
                        op0=bypass, op1=mult, accum_out=S[:I, 10:11],
                    )
                    nc.vector.scalar_tensor_tensor(
                        out=scr_i[:, :], in0=G_sb[:, :], scalar=1.0, in1=G_sb[:, :],
                        op0=bypass, op1=mult, accum_out=S[:I, 11:12],
                    ).then_inc(s_vfin, 1)
            # tail: subtract the CB part first (ACT squares it), then the CA
            # part which is squared here so the tail has no extra engine hop
            vector.wait_ge(s_x[LAST], 32)
            nc.vector.tensor_sub(
                xb[LAST][:, 0:CB], xb[LAST][:, 0:CB], db[LAST][:, 0:CB]
            ).then_inc(s_sub, 1)
            nc.vector.tensor_sub(
                xb[LAST][:, CB:], xb[LAST][:, CB:], db[LAST][:, CB:]
            )
            nc.vector.scalar_tensor_tensor(
                out=scr_e[:, 0:CA], in0=xb[LAST][:, CB:], scalar=1.0,
                in1=xb[LAST][:, CB:], op0=bypass, op1=mult,
                accum_out=S[:, 6:7],
            ).then_inc(s_vfin, 1)

        @block.tensor
        def _(tensor):
            tensor.wait_ge(s_packE, 16)
            for t in range(RT):
                nc.tensor.matmul(
                    psum_M[:, :], lhsT=enc_t(t), rhs=lat_t(t),
                    start=(t == 0), stop=(t == RT - 1),
                )
            for t in range(RT):
                nc.tensor.matmul(
                    psum_L[:, :], lhsT=lat_t(t), rhs=lat_t(t),
                    start=(t == 0), stop=(t == RT - 1),
                )
            nc.tensor.matmul(
                psum_G[:, :], lhsT=rsra_sb, rhs=rsra_sb, start=True, stop=True
            ).then_inc(s_pe, 1)

    return nc


def kernel(x, encoded, latent, decoded, rsrA):
    global _NC, LAST_RESULT
    if _NC is None:
        _NC = _build_nc()

    x = np.ascontiguousarray(x, dtype=np.float32)
    decoded = np.ascontiguousarray(decoded, dtype=np.float32)
    encoded = np.ascontiguousarray(encoded, dtype=np.float32)
    latent = np.ascontiguousarray(latent, dtype=np.float32)
    rsrA = np.ascontiguousarray(rsrA, dtype=np.float32)

    in_maps = []
    for c in range(N_CORES):
        sl = slice(c * R, (c + 1) * R)
        pk = np.concatenate(
            [
                encoded[sl].reshape(P, ENC_W),
                latent[sl].reshape(P, LAT_W),
                rsrA,
            ],
            axis=1,
        )
        in_maps.append({"x": x[sl], "dec": decoded[sl], "pack": pk})

    res = run_bass_kernel_spmd(_NC, in_maps, core_ids=list(range(N_CORES)), trace=TRACE)
    LAST_RESULT = res

    o = np.stack([r["out"] for r in res.results]).astype(np.float64)  # [8,128,16]
    cols = o.sum(axis=(0, 1))  # [16]
    # cols 0..4: tile row-sums of (x-dec)^2, col 5: tail-B, col 6: tail-A,
    # col 7: enc^2, col 8: rsrA^2, col 9: cross, col 10: zsq, col 11: G^2
    s_recon = cols[0:7].sum()
    s_enc2 = cols[7]
    s_cross = cols[9]
    s_zsq = cols[10]
    g2 = o[0, :, 11].sum()
    ra2 = o[0, :, 8].sum()

    pca_sq = s_enc2 - 2.0 * s_cross + s_zsq
    proj_sq = g2 - 2.0 * ra2 + float(I)
    loss = s_recon / B + 1.1 * pca_sq / B + 0.1 * proj_sq / (I * I)
    return np.asarray(loss, dtype=np.float32)


# revision 23
# speedup vs baseline: 1.0545x; 1.0545x over previous
"""Trainium2 Bass kernel for the LELoss problem (raw Bass, 8-core SPMD).

loss = mean_b ||x_b - dec_b||^2
     + 1.1 * mean_b ||enc_b - (lat @ rsrA.T)_b||^2
     + 0.1 * mean((rsrA.T @ rsrA - I)^2)

(The knn/cdist/topk in the original module is dead code - its result is never
used - so the returned loss reduces to the three terms above.)

Per-core algebra (batch shard of R=1024 rows):
  sum||enc - lat@A.T||^2 = sum(enc^2) - 2*sum(M .* A) + sum(L .* G0)
      with M = enc.T @ lat [E,I], L = lat.T @ lat [I,I], G0 = A.T @ A [I,I]
  sum((G0 - I)^2) = sum(G0^2) - 2*sum(A^2) + I_dim
All partial sums land in columns of a [128,16] SBUF accumulator S which is
DMA'd out per core; the host collapses partitions/cores and applies weights.

DMA strategy (v7): ONE HWDGE ring (SP) carries the whole stream - the 16
SDMA engines hit the same ~427GB/s from one ring as from two, completion
sems fire in strict FIFO right behind the data, and the ACT engine is freed
for compute. Only [128, C] tiles (other partition counts break the
16-engine descriptor spray). Pair-adjacent order x_t, dec_t: three
[128,2048] row-pair-packed tiles (8KB descriptors, which also evens out the
~12% slower SDMA engine 15), the enc/lat pack right after the first pair
(so the PE matmuls and reductions run mid-stream), then a [128,1024] pair
and two [128,512] column-half pairs so the post-stream tail is only ~1.1us
(subtract 320/192-split between DVE and ACT). A dummy 1-element activation
pre-triggers the 1.3us ACT table load, and the out DMA's completion is left
to the NEFF teardown (~7us sem-reset postamble) instead of a semaphore wait.
"""

import contextlib

import numpy as np

try:
    import concourse.bass as bass
except ImportError:  # pragma: no cover - grading env fallback
    import sys

    sys.path.insert(0, "/opt/trn_rl_repo")
    import concourse.bass as bass

from concourse import mybir
from concourse.bass_utils import run_bass_kernel_spmd

N_CORES = 8
B, D, E, I = 8192, 1024, 128, 20
R = B // N_CORES  # rows per core = 1024
P = 128  # SBUF partitions
S_COLS = 16
F32 = mybir.dt.float32

RT = R // P  # enc pack row groups = 8
ENC_W = RT * E  # 1024 cols of packed enc
LAT_W = RT * I  # 160 cols of packed lat
PACK_W = ENC_W + LAT_W + I  # 1204

# stream tiles: (row0, nrows, col0, ncols). 256-row tiles land as
# [128, 2048] row-pair-packed views with 8KB descriptors.
TILES = [
    (0, 256, 0, D),      # [128,2048]
    (256, 256, 0, D),    # [128,2048]
    (512, 256, 0, D),    # [128,2048]
    (768, 128, 0, D),    # [128,1024]
    (896, 128, 0, 512),  # [128,512]
    (896, 128, 512, 512),  # [128,512]
]
NT = len(TILES)
LAST = NT - 1
SQ_TOTAL = NT + 2  # per-tile squares (NT-1 + tail-B) + enc^2 + rsrA^2

# column split of the final [128,512] tile: ACT squares CB cols,
# DVE STT-squares the remaining CA cols (DVE also does both subtracts)
CB = 320
CA = 512 - CB  # 192

WAIT_OUT = False  # wait for the out DMA's completion semaphore before ending

TRACE = False
LAST_RESULT = None

_NC = None


def _build_nc():
    nc = bass.Bass()
    x = nc.dram_tensor("x", [R, D], F32, kind="ExternalInput")
    dec = nc.dram_tensor("dec", [R, D], F32, kind="ExternalInput")
    pack = nc.dram_tensor("pack", [P, PACK_W], F32, kind="ExternalInput")
    out = nc.dram_tensor("out", [P, S_COLS], F32, kind="ExternalOutput")

    Square = mybir.ActivationFunctionType.Square
    mult = mybir.AluOpType.mult
    bypass = mybir.AluOpType.bypass

    def tile_src(dram, t):
        r0, rn, c0, cn = TILES[t]
        ap = dram[r0 : r0 + rn, c0 : c0 + cn]
        if rn == 2 * P:
            ap = ap.rearrange("(p two) d -> p (two d)", two=2)
        return ap

    def sbuf_cols(t):
        r0, rn, c0, cn = TILES[t]
        return cn * (rn // P)

    ctx = contextlib.ExitStack()
    with ctx:
        xb = [
            ctx.enter_context(nc.sbuf_tensor(f"xb{t}", [P, sbuf_cols(t)], F32))
            for t in range(NT)
        ]
        db = [
            ctx.enter_context(nc.sbuf_tensor(f"db{t}", [P, sbuf_cols(t)], F32))
            for t in range(NT)
        ]
        small_sb = ctx.enter_context(nc.sbuf_tensor([P, PACK_W], F32))
        S = ctx.enter_context(nc.sbuf_tensor([P, S_COLS], F32))
        G_sb = ctx.enter_context(nc.sbuf_tensor([I, I], F32))
        scr_m = ctx.enter_context(nc.sbuf_tensor([E, I], F32))
        scr_i = ctx.enter_context(nc.sbuf_tensor([I, I], F32))
        scr_a = ctx.enter_context(nc.sbuf_tensor([E, I], F32))
        scr_e = ctx.enter_context(nc.sbuf_tensor([P, ENC_W], F32))
        dummy = ctx.enter_context(nc.sbuf_tensor([P, 2], F32))

        psum_M = ctx.enter_context(nc.psum_tensor([E, I], F32))
        psum_L = ctx.enter_context(nc.psum_tensor([I, I], F32))
        psum_G = ctx.enter_context(nc.psum_tensor([I, I], F32))

        # pair sems: tile t complete when s_x[t] >= 32 (16 from x, 16 from dec)
        s_x = [ctx.enter_context(nc.semaphore(f"s_x{t}")) for t in range(NT)]
        s_packE = ctx.enter_context(nc.semaphore("s_packE"))
        s_init = ctx.enter_context(nc.semaphore("s_init"))
        s_sub = ctx.enter_context(nc.semaphore("s_sub"))
        s_sq = ctx.enter_context(nc.semaphore("s_sq"))
        s_pe = ctx.enter_context(nc.semaphore("s_pe"))
        s_vfin = ctx.enter_context(nc.semaphore("s_vfin"))
        s_out = ctx.enter_context(nc.semaphore("s_out"))

        block = ctx.enter_context(nc.Block())

        def enc_t(t):
            return small_sb[:, t * E : (t + 1) * E]

        def lat_t(t):
            return small_sb[:, ENC_W + t * I : ENC_W + (t + 1) * I]

        rsra_sb = small_sb[:, ENC_W + LAT_W : PACK_W]

        @block.sync
        def _(sync):
            def pair(t):
                sync.dma_start(out=xb[t][:, :], in_=tile_src(x, t)).then_inc(
                    s_x[t], 16
                )
                sync.dma_start(out=db[t][:, :], in_=tile_src(dec, t)).then_inc(
                    s_x[t], 16
                )

            pair(0)
            sync.dma_start(out=small_sb[:, :], in_=pack[:, :]).then_inc(s_packE, 16)
            for t in range(1, NT):
                pair(t)


        @block.scalar
        def _(scalar):
            # pre-trigger the ACT function-table load while the stream ramps
            nc.scalar.activation(out=dummy[:, 0:1], in_=dummy[:, 1:2], func=Square)
            # squares of the streamed differences (tiles 0..NT-2)
            scalar.wait_ge(s_init, 1)
            for t in range(NT - 1):
                scalar.wait_ge(s_sub, t + 1)
                nc.scalar.activation(
                    out=db[t][:, :], in_=xb[t][:, :], func=Square,
                    accum_out=S[:, t : t + 1],
                ).then_inc(s_sq, 1)
                if t == 1:
                    scalar.wait_ge(s_packE, 16)
                    nc.scalar.activation(
                        out=scr_e[:, :], in_=small_sb[:, 0:ENC_W], func=Square,
                        accum_out=S[:, 7:8],
                    ).then_inc(s_sq, 1)
                    nc.scalar.activation(
                        out=scr_a[:, :], in_=rsra_sb, func=Square,
                        accum_out=S[:E, 8:9],
                    ).then_inc(s_sq, 1)
            # tail: square the CB-column part as soon as the DVE subtracted it
            scalar.wait_ge(s_sub, NT)
            nc.scalar.activation(
                out=db[LAST][:, 0:CB], in_=xb[LAST][:, 0:CB], func=Square,
                accum_out=S[:, 5:6],
            ).then_inc(s_sq, 1)
            # ship the accumulator. The read-accumulator micro-op that
            # writes each S column overlaps the engine's next instruction,
            # so wait on s_sq (which fires at read-accum completion) for the
            # ACT columns - including this engine's own tail-B - and on
            # s_vfin for the DVE columns.
            scalar.wait_ge(s_sq, SQ_TOTAL)
            scalar.wait_ge(s_vfin, 2)
            scalar.dma_start(out=out[:, :], in_=S[:, :]).then_inc(s_out, 16)
            if WAIT_OUT:
                scalar.wait_ge(s_out, 16)

        @block.vector
        def _(vector):
            nc.vector.memset(S[:, :], 0.0).then_inc(s_init, 1)
            # the big stream: d = x - dec, in place
            for t in range(NT - 1):
                vector.wait_ge(s_x[t], 32)
                nc.vector.tensor_sub(xb[t][:, :], xb[t][:, :], db[t][:, :]).then_inc(
                    s_sub, 1
                )
                if t == 1:
                    # tiny fused reductions over the PCA/proj matmul results,
                    # mid-stream in a gap between subtracts
                    vector.wait_ge(s_pe, 1)
                    nc.vector.tensor_copy(G_sb[:, :], psum_G[:, :])
                    nc.vector.scalar_tensor_tensor(
                        out=scr_m[:, :], in0=psum_M[:, :], scalar=1.0, in1=rsra_sb,
                        op0=bypass, op1=mult, accum_out=S[:E, 9:10],
                    )
                    nc.vector.scalar_tensor_tensor(
                        out=scr_i[:, :], in0=psum_L[:, :], scalar=1.0, in1=G_sb[:, :],
                        op0=bypass, op1=mult, accum_out=S[:I, 10:11],
                    )
                    nc.vector.scalar_tensor_tensor(
                        out=scr_i[:, :], in0=G_sb[:, :], scalar=1.0, in1=G_sb[:, :],
                        op0=bypass, op1=mult, accum_out=S[:I, 11:12],
                    ).then_inc(s_vfin, 1)
            # tail: subtract the CB part first (ACT squares it), then the CA
            # part which is squared here so the tail has no extra engine hop
            vector.wait_ge(s_x[LAST], 32)
            nc.vector.tensor_sub(
                xb[LAST][:, 0:CB], xb[LAST][:, 0:CB], db[LAST][:, 0:CB]
            ).then_inc(s_sub, 1)
            nc.vector.tensor_sub(
                xb[LAST][:, CB:], xb[LAST][:, CB:], db[LAST][:, CB:]
            )
            nc.vector.scalar_tensor_tensor(
                out=scr_e[:, 0:CA], in0=xb[LAST][:, CB:], scalar=1.0,
                in1=xb[LAST][:, CB:], op0=bypass, op1=mult,
                accum_out=S[:, 6:7],
            ).then_inc(s_vfin, 1)

        @block.tensor
        def _(tensor):
            tensor.wait_ge(s_packE, 16)
            for t in range(RT):
                nc.tensor.matmul(
                    psum_M[:, :], lhsT=enc_t(t), rhs=lat_t(t),
                    start=(t == 0), stop=(t == RT - 1),
                )
            for t in range(RT):
                nc.tensor.matmul(
                    psum_L[:, :], lhsT=lat_t(t), rhs=lat_t(t),
                    start=(t == 0), stop=(t == RT - 1),
                )
            nc.tensor.matmul(
                psum_G[:, :], lhsT=rsra_sb, rhs=rsra_sb, start=True, stop=True
            ).then_inc(s_pe, 1)

    return nc


def kernel(x, encoded, latent, decoded, rsrA):
    global _NC, LAST_RESULT
    if _NC is None:
        _NC = _build_nc()

    x = np.ascontiguousarray(x, dtype=np.float32)
    decoded = np.ascontiguousarray(decoded, dtype=np.float32)
    encoded = np.ascontiguousarray(encoded, dtype=np.float32)
    latent = np.ascontiguousarray(latent, dtype=np.float32)
    rsrA = np.ascontiguousarray(rsrA, dtype=np.float32)

    in_maps = []
    for c in range(N_CORES):
        sl = slice(c * R, (c + 1) * R)
        pk = np.concatenate(
            [
                encoded[sl].reshape(P, ENC_W),
                latent[sl].reshape(P, LAT_W),
                rsrA,
            ],
            axis=1,
        )
        in_maps.append({"x": x[sl], "dec": decoded[sl], "pack": pk})

    res = run_bass_kernel_spmd(_NC, in_maps, core_ids=list(range(N_CORES)), trace=TRACE)
    LAST_RESULT = res

    o = np.stack([r["out"] for r in res.results]).astype(np.float64)  # [8,128,16]
    cols = o.sum(axis=(0, 1))  # [16]
    # cols 0..4: tile row-sums of (x-dec)^2, col 5: tail-B, col 6: tail-A,
    # col 7: enc^2, col 8: rsrA^2, col 9: cross, col 10: zsq, col 11: G^2
    s_recon = cols[0:7].sum()
    s_enc2 = cols[7]
    s_cross = cols[9]
    s_zsq = cols[10]
    g2 = o[0, :, 11].sum()
    ra2 = o[0, :, 8].sum()

    pca_sq = s_enc2 - 2.0 * s_cross + s_zsq
    proj_sq = g2 - 2.0 * ra2 + float(I)
    loss = s_recon / B + 1.1 * pca_sq / B + 0.1 * proj_sq / (I * I)
    return np.asarray(loss, dtype=np.float32)
